# revision 40
# baseline (speedup 1.0000x reference)
"""MatchNet kernel for 8 Trainium2 NeuronCores.

Math (per batch b):
    keys   = q[b] @ W + bias
    scores = p[b] @ keys^T
    attn   = softmax(scores, axis=-1)
    out[b] = relu(attn @ q[b])

Because softmax is shift-invariant along the reduced axis, the Dense bias
contributes p@b^T (constant along lq) to scores and therefore has NO effect
on the output — it is dropped entirely.

Sharding: data-parallel over B=16 across 8 cores (2 batches per core).
W is broadcast. p and q are transposed on the host so every on-chip matmul
has its contraction dim on SBUF partitions.

Score path (3 matmul passes instead of the previous 6): associativity is
used to fold W onto p instead of q:
    scores = p @ (qW)^T = (p @ W^T) @ q^T = G @ q^T
with asymmetric precision (PE takes mixed bf16/fp16 operands; products are
exact into fp32 PSUM):
    MM1: GT[h, lp] = sum_hk (Wb+Wr)[hk, h] * pT16[hk, lp]   2 bf16xfp16 passes
         (Wb, Wr) = bf16 split of W^T (host), pT16 = fp16(p^T) (host);
         the W pair keeps G's computation error ~2^-13 (it amplifies by
         sqrt(H)*|p| ~ 32 in the scores), while p's own fp16 rounding is
         the accepted error floor.
    MM2: scores[lp, lq] = sum_h G16[h, lp] * qT16[h, lq]   1 fp16 pass
         (G16 = fp16 of the PSUM G, single DVE copy per h-tile)
    softmax over free dim; exp via ACT (bias=-rowmax, accum rowsum),
    exp output stored fp16
    T:   attnT[lq, lp] via PE transpose (fp16)
    MM3: out[lp, h] = sum_lq attnT[lq, lp] * qn16[lq, h]  single fp16 pass
    relu(out * (1/rowsum)) via ACT with per-partition scale, stored fp16
    (upcast to fp32 on host)
Error: the three fp16 roundings (p, G-storage, q) each contribute ~7-9e-3
of max-rel error through the sharp softmax; HW-measured total is 1.406e-2
vs the 2e-2 gate, bit-stable across runs (inputs and the instruction
stream are deterministic). A 4-pass variant (G kept as a bf16 pair,
2-pass MM2) measures 1.05e-2 at +55us if more margin is ever needed; the
old 6-pass kernel measured 3.75e-4 at +175us. A numpy simulator of these
exact schemes on the harness inputs predicts all of them to ~1e-3
(sim 1.27e-2 / 9.4e-3 / 3.76e-4).
Perf notes (HW-measured):
  - ~256-259us vs 432us for the 6-pass kernel; PE-bound (~94% active), at
    the PE-work floor of this pass structure (~235us stream+startup/drain).
  - Warmup is 45 zero-matmuls, ending ~4.5us BEFORE the first inputs land:
    the idle+HAM-recool window is deliberate. Warmup long enough to make PE
    activity fully continuous from t=0 made runs 25% SLOWER chip-wide (all
    engines ~20% down) — it appears to trip a power-state downclock (P0,
    ~2.0GHz). Do not "fix" the startup gap by extending warmup.
  - Each dma_start costs ~600ns of serial SP issue time starting ~6.7us in,
    and THAT (not transfer bandwidth — transfers stripe across 16 DMA
    engines) paces when compute can start. Hence chunk-PAIR loads (KP=2);
    splitting loads finer made startup +12us worse. Issuing DMAs from other
    engines also loses: gpsimd takes ~2-3us per dma_start, scalar's hwdge
    queue only comes up ~14us in.
  - DMA xbar transpose for attnT was tried instead of PE transposes and
    was ~110us slower end-to-end (xbar-mode serialization). Splitting the
    last tile into 64-row halves to pipeline its drain was ~10us slower:
    64-partition outputs halve PE efficiency (same streamed columns, half
    the array rows working).
  - exp/stats are issued one pipeline stage ahead of the transposes so the
    last tile's drain chain starts at its transposes, and the last tile
    stores through 4 relu chunks x 2 parallel DMAs (the final store's DMA
    is on the critical path before the end-of-kernel barrier).
"""

import os
from contextlib import ExitStack

import ml_dtypes
import numpy as np

import concourse.bass as bass
import concourse.mybir as mybir
import concourse.tile as tile
from concourse import bacc
from concourse.bass import ts
from concourse.bass_utils import run_bass_kernel_spmd
from concourse.masks import make_identity

B, L, H = 16, 1024, 1024
NCORES = 8
BPC = B // NCORES  # batches per core
P = 128
KO = H // P        # 8 contraction chunks
NT = L // P        # 8 lp tiles per batch
NF = 512           # matmul moving free dim
NCH = L // NF      # 2 free chunks
F32 = mybir.dt.float32
BF16 = mybir.dt.bfloat16
FP16 = mybir.dt.float16
AF = mybir.ActivationFunctionType
AX = mybir.AxisListType


def _build_body(ctx, tc, ins, out):
    nc = tc.nc
    pT16, qT16, qn16, Wb, Wr = ins

    # PE warmup: the first ~15us are DMA-bound (bootstrap + first loads) and
    # the PE would sit idle, entering the kernel HAM-throttled at 1.2 GHz.
    # Zero matmuls during that window cost nothing and flip the clock
    # gate to 2.4 GHz before the real matmuls start.
    with (
        tc.tile_pool(name="warm", bufs=1) as warm_pool,
        tc.tile_pool(name="warmps", bufs=1, space=bass.MemorySpace.PSUM) as wps_pool,
    ):
        wsb = warm_pool.tile([P, P], BF16)
        nc.gpsimd.memset(wsb[:], 0.0)
        wps = wps_pool.tile([P, P], F32)
        for _ in range(45):
            nc.tensor.matmul(wps[:], wsb[:], wsb[:], start=True, stop=True)

    const = ctx.enter_context(tc.tile_pool(name="const", bufs=1))
    # W splits as single 3D tiles, loaded in chunk-PAIR DMAs: each dma_start
    # costs ~600ns of serial SP issue time, so fewer, 2-chunk loads start
    # the last critical transfer ~3us earlier while slice-level deps keep
    # pair granularity for the first matmuls.
    Wb_sb = const.tile([P, KO, H], BF16, name="Wb_sb")
    Wr_sb = const.tile([P, KO, H], BF16, name="Wr_sb")
    ident = const.tile([P, P], FP16)
    make_identity(nc, ident[:])

    pT_pool = ctx.enter_context(tc.tile_pool(name="pTp", bufs=2))
    qT_pool = ctx.enter_context(tc.tile_pool(name="qTp", bufs=2))
    q_pool = ctx.enter_context(tc.tile_pool(name="qp", bufs=2))
    gT_pool = ctx.enter_context(tc.tile_pool(name="gTp", bufs=1))
    attn_pool = ctx.enter_context(tc.tile_pool(name="attnp", bufs=2))
    attnT_pool = ctx.enter_context(tc.tile_pool(name="attnTp", bufs=2))
    osb_pool = ctx.enter_context(tc.tile_pool(name="osbp", bufs=2))
    stat_pool = ctx.enter_context(tc.tile_pool(name="statp", bufs=8))
    ps_big = ctx.enter_context(
        tc.tile_pool(name="psbig", bufs=3, space=bass.MemorySpace.PSUM)
    )
    ps_t = ctx.enter_context(
        tc.tile_pool(name="pst", bufs=2, space=bass.MemorySpace.PSUM)
    )

    Wb_re = Wb.rearrange("(ko ki) h -> ki ko h", ki=P)
    Wr_re = Wr.rearrange("(ko ki) h -> ki ko h", ki=P)

    # Pre-issue ALL per-batch input loads, spread across engines. The SP
    # (sync) engine needs ~600ns per dma_start and only reaches them ~6.7us
    # in, so issuing the phase-1-critical loads (pT b0 / Wb / Wr) from the
    # otherwise-idle gpsimd/scalar/vector engines (alive ~3.4us) starts the
    # transfers ~5us earlier; the transfers themselves stripe across 16 DMA
    # engines and are not the bottleneck. Hoisting batch 1's loads here also
    # keeps them ahead of batch 0's semaphore-paced out-store triggers in
    # the SP FIFO.
    pT_sb_all = [
        pT_pool.tile([P, KO, L], FP16, name=f"pT_sb_{b}", tag="pT_sb")
        for b in range(BPC)
    ]
    qT_sb_all = [
        qT_pool.tile([P, KO, L], FP16, name=f"qT_sb_{b}", tag="qT_sb")
        for b in range(BPC)
    ]
    # All loads issue from SP: scalar/gpsimd-issued DMAs were tried and are
    # WORSE (gpsimd ~2-3us per dma_start; scalar's hwdge queue comes up only
    # ~14us in, stalling phase 1 on W).
    KP = 2  # chunks per dma_start
    pT_re0 = pT16[0].rearrange("(ko ki) l -> ki ko l", ki=P)
    for j in range(KO // KP):
        nc.sync.dma_start(Wb_sb[:, ts(j, KP), :], Wb_re[:, ts(j, KP), :])
        nc.sync.dma_start(pT_sb_all[0][:, ts(j, KP), :], pT_re0[:, ts(j, KP), :])
    for j in range(KO // KP):
        nc.sync.dma_start(Wr_sb[:, ts(j, KP), :], Wr_re[:, ts(j, KP), :])
    qT_re0 = qT16[0].rearrange("(ko ki) l -> ki ko l", ki=P)
    for j in range(KO // KP):
        nc.sync.dma_start(qT_sb_all[0][:, ts(j, KP), :], qT_re0[:, ts(j, KP), :])
    for b in range(1, BPC):
        pT_re = pT16[b].rearrange("(ko ki) l -> ki ko l", ki=P)
        qT_re = qT16[b].rearrange("(ko ki) l -> ki ko l", ki=P)
        for j in range(KO // KP):
            nc.sync.dma_start(pT_sb_all[b][:, ts(j, KP), :], pT_re[:, ts(j, KP), :])
        for j in range(KO // KP):
            nc.sync.dma_start(qT_sb_all[b][:, ts(j, KP), :], qT_re[:, ts(j, KP), :])

    for b in range(BPC):
        pT_sb = pT_sb_all[b]
        qT_sb = qT_sb_all[b]

        # ---- phase 1: GT[h, lp] = (p @ W^T)^T, 2-pass asym, stored fp16
        g16_sb = gT_pool.tile([P, KO, L], FP16, name=f"g16_{b}", tag="g16")
        mm1_passes = (Wb_sb, Wr_sb)
        for m in range(KO):
            ps_k = ps_big.tile([P, L], F32, name=f"ps_k_{b}_{m}", tag="ps_big")
            for n in range(NCH):
                for pi, Asb in enumerate(mm1_passes):
                    for k in range(KO):
                        nc.tensor.matmul(
                            ps_k[:, ts(n, NF)],
                            Asb[:, k, ts(m, P)],
                            pT_sb[:, k, ts(n, NF)],
                            start=(pi == 0 and k == 0),
                            stop=(pi == len(mm1_passes) - 1 and k == KO - 1),
                        )
            nc.vector.tensor_copy(g16_sb[:, m, :], ps_k[:])

        # q natural (fp16, for MM3): issued after phase-1 compute so its DMA
        # queues drain behind the phase-1-critical loads.
        qn_sb = q_pool.tile([P, KO, H], FP16, name=f"qn_sb_{b}", tag="qn_sb")
        qre = qn16[b].rearrange("(ko ki) h -> ki ko h", ki=P)
        for k in range(KO):
            nc.sync.dma_start(qn_sb[:, k, :], qre[:, k, :])

        # ---- phase 2/3: per lp tile, software-pipelined
        scores_ps = {}
        soft = {}

        def stage_scores(i, b=b, g16_sb=g16_sb, qT_sb=qT_sb):
            ps_s = ps_big.tile([P, L], F32, name=f"ps_s_{b}_{i}", tag="ps_big")
            for n in range(NCH):
                for k in range(KO):
                    nc.tensor.matmul(
                        ps_s[:, ts(n, NF)],
                        g16_sb[:, k, ts(i, P)],
                        qT_sb[:, k, ts(n, NF)],
                        start=(k == 0),
                        stop=(k == KO - 1),
                    )
            scores_ps[i] = ps_s

        exp_out = {}

        def stage_exp(i, b=b):
            # issued one pipeline stage early so the ACT exp and DVE stats of
            # tile i hide under tile i-1's PE work (matters for the last tile,
            # whose drain chain otherwise starts with an exposed exp).
            ps_s = scores_ps.pop(i)
            negmax = stat_pool.tile([P, 1], F32, name=f"negmax_{b}_{i}", tag="negmax")
            nc.vector.reduce_max(negmax[:], ps_s[:], axis=AX.X, negate=True)
            attn_sb = attn_pool.tile([P, L], FP16, name=f"attn_{b}_{i}", tag="attn")
            rowsum = stat_pool.tile([P, 1], F32, name=f"rowsum_{b}_{i}", tag="rowsum")
            nc.scalar.activation(
                attn_sb[:],
                ps_s[:],
                AF.Exp,
                bias=negmax[:],
                accum_out=rowsum[:],
            )
            recip = stat_pool.tile([P, 1], F32, name=f"recip_{b}_{i}", tag="recip")
            nc.vector.reciprocal(recip[:], rowsum[:])
            exp_out[i] = (attn_sb, recip)

        def stage_transpose(i, b=b):
            attn_sb, recip = exp_out.pop(i)
            attnT_sb = attnT_pool.tile([P, L], FP16, name=f"attnT_{b}_{i}", tag="attnT")
            for g in range(L // NF):
                ps_tt = ps_t.tile([P, NF], FP16, name=f"ps_tt_{b}_{i}_{g}", tag="ps_t")
                for j in range(NF // P):
                    c = g * (NF // P) + j
                    nc.tensor.transpose(
                        ps_tt[:, ts(j, P)], attn_sb[:, ts(c, P)], ident[:]
                    )
                nc.vector.tensor_copy(attnT_sb[:, ts(g, NF)], ps_tt[:])
            soft[i] = (attnT_sb, recip)

        def stage_mm3(i, b=b, qn_sb=qn_sb, fine=False):
            attnT_sb, recip = soft.pop(i)
            out_sb = osb_pool.tile([P, H], FP16, name=f"out_sb_{b}_{i}", tag="out_sb")
            ps_o = ps_big.tile([P, H], F32, name=f"ps_o_{b}_{i}", tag="ps_big")
            # relu+store per n-chunk so the drain of chunk 0 hides under the
            # matmuls of chunk 1 (shrinks the kernel tail). The last tile
            # uses 4 finer chunks to shrink the final relu+DMA exposure.
            nf = NF // 2 if fine else NF
            if not fine:
                # k-outer: the k<4 matmuls only need the first attnT
                # half-copy, so the PE starts while the DVE writes the
                # second half (removes ~0.7us stalls in scores-free late
                # iterations); also one LDWEIGHTS per k instead of two.
                for k in range(KO):
                    for n in range(H // nf):
                        nc.tensor.matmul(
                            ps_o[:, ts(n, nf)],
                            attnT_sb[:, ts(k, P)],
                            qn_sb[:, k, ts(n, nf)],
                            start=(k == 0),
                            stop=(k == KO - 1),
                        )
            for n in range(H // nf):
                if fine:
                    for k in range(KO):
                        nc.tensor.matmul(
                            ps_o[:, ts(n, nf)],
                            attnT_sb[:, ts(k, P)],
                            qn_sb[:, k, ts(n, nf)],
                            start=(k == 0),
                            stop=(k == KO - 1),
                        )
                nc.scalar.activation(
                    out_sb[:, ts(n, nf)], ps_o[:, ts(n, nf)], AF.Relu, scale=recip[:]
                )
                if fine:
                    # two parallel half-chunk stores: the final store's DMA
                    # transfer is on the kernel's critical path.
                    h2 = nf // 2
                    nc.sync.dma_start(
                        out[b, ts(i, P), ts(2 * n, h2)], out_sb[:, ts(2 * n, h2)]
                    )
                    nc.sync.dma_start(
                        out[b, ts(i, P), ts(2 * n + 1, h2)],
                        out_sb[:, ts(2 * n + 1, h2)],
                    )
                else:
                    nc.sync.dma_start(
                        out[b, ts(i, P), ts(n, nf)], out_sb[:, ts(n, nf)]
                    )

        last = b == BPC - 1
        stage_scores(0)
        stage_scores(1)
        stage_exp(0)
        for i in range(NT):
            stage_transpose(i)
            if i + 2 < NT:
                stage_scores(i + 2)
            if i + 1 < NT:
                stage_exp(i + 1)
            stage_mm3(i, fine=last and i == NT - 1)


_IN_NAMES = ["pT16", "qT16", "qn16", "Wb", "Wr"]

_CACHED = None


def _get_program():
    global _CACHED
    if _CACHED is not None:
        return _CACHED
    nc = bacc.Bacc(
        "TRN2",
        target_bir_lowering=False,
        debug=False,
        num_devices=NCORES,
    )
    specs = {
        "pT16": ([BPC, H, L], FP16),
        "qT16": ([BPC, H, L], FP16),
        "qn16": ([BPC, L, H], FP16),
        "Wb": ([H, H], BF16),
        "Wr": ([H, H], BF16),
    }
    handles = [
        nc.dram_tensor(name, *specs[name], kind="ExternalInput") for name in _IN_NAMES
    ]
    out_h = nc.dram_tensor("out", [BPC, L, H], FP16, kind="ExternalOutput")
    with tile.TileContext(nc) as tc:
        with ExitStack() as ctx:
            _build_body(ctx, tc, [h.ap() for h in handles], out_h.ap())
    nc.compile()
    _CACHED = nc
    return nc


def _split_bf16(x):
    xb = x.astype(ml_dtypes.bfloat16)
    xr = (x - xb.astype(np.float32)).astype(ml_dtypes.bfloat16)
    return xb, xr


def kernel(p, q, W_key, b_key):
    # b_key is mathematically irrelevant: softmax over lq is invariant to the
    # per-lp constant p@b^T it adds to scores, and keys are not used elsewhere.
    del b_key
    p = np.ascontiguousarray(np.asarray(p, dtype=np.float32))
    q = np.ascontiguousarray(np.asarray(q, dtype=np.float32))
    W = np.ascontiguousarray(np.asarray(W_key, dtype=np.float32))

    pT16 = np.ascontiguousarray(p.transpose(0, 2, 1)).astype(np.float16)
    qT16 = np.ascontiguousarray(q.transpose(0, 2, 1)).astype(np.float16)
    qn16 = q.astype(np.float16)
    # MM1 contracts over W's OUTPUT dim (G = p @ W^T), so the stationary
    # operand layout is [h_out (contraction), h_in] = W transposed.
    Wb, Wr = _split_bf16(np.ascontiguousarray(W.T))
    full = {"pT16": pT16, "qT16": qT16, "qn16": qn16}

    in_maps = []
    for c in range(NCORES):
        sl = slice(c * BPC, (c + 1) * BPC)
        m = {k: np.ascontiguousarray(v[sl]) for k, v in full.items()}
        m["Wb"] = Wb
        m["Wr"] = Wr
        in_maps.append(m)

    nc = _get_program()
    trace = bool(int(os.environ.get("MATCHNET_TRACE", "0")))
    res = run_bass_kernel_spmd(nc, in_maps, list(range(NCORES)), trace=trace)
    if trace:
        kernel.last_exec_time_ns = res.exec_time_ns
        kernel.last_results = res
    # out is stored fp16 on-device (halves the final DMA drain); upcast here.
    out = np.concatenate(
        [res.results[c]["out"].astype(np.float32) for c in range(NCORES)], axis=0
    )
    return out


kernel.last_exec_time_ns = None
kernel.last_results = None
